# revision 14
# baseline (speedup 1.0000x reference)
"""AELoss on 8 TRN2 NeuronCores — visible-packed gather, lean tail.

Front end matches the proven layout: visible joints packed into a
[128, 9] slot grid, one indirect DMA per column (the ~1.04us/instruction
SWDGE descriptor pass serializes on GpSimd and dominates). Everything
else is streamlined around it:
- GpSimd runs ONLY the 9 gathers; iota/identity/selector constants ship
  from the host in one DMA, so descriptor gen starts as soon as the
  offsets land.
- Per-person visible counts also ship from host, so the valid/selector
  algebra runs during the gathers and the accumulating PE matmuls carry
  just 2 columns (sum g, sum g^2) instead of 3.
- The pairwise push field folds its mask and row reduction through one
  PE matmul (pm @ vb, pm is symmetric) and one fused multiply+row-sum.
- The device ships per-image (pull_sum, num_tags, push_tot); the final
  normalization of 4 scalars per core happens on host.
"""

import numpy as np

B = 32
N = 17 * 256 * 256  # 1114112 flattened tag-map size
P = 30              # max people per image
J = 17              # joints per person
M = 8               # cores
BL = B // M         # images per core = 4
PART = BL * P       # person partitions = 120
C = 9               # packed gather columns (capacity 128*9 = 1152 slots)
LROWS = 32          # live rows in the last gather column (max 1054-1024=30)
CW = 1 + BL + PART  # consts width: cnt | sel | identity

_CACHE = {}


def _build():
    from contextlib import ExitStack

    import concourse.bass as bass
    import concourse.tile as tile
    from concourse import bacc, mybir

    f32 = mybir.dt.float32
    i32 = mybir.dt.int32
    Alu = mybir.AluOpType

    nc = bacc.Bacc("TRN2", target_bir_lowering=False, debug=False)

    tags = nc.dram_tensor("tags", [BL * N, 1], f32, kind="ExternalInput")
    joff = nc.dram_tensor("joff", [128, C], i32, kind="ExternalInput")
    jmask = nc.dram_tensor("jmask", [128, C * PART], f32, kind="ExternalInput")
    cst = nc.dram_tensor("cst", [PART, CW], f32, kind="ExternalInput")
    out = nc.dram_tensor("out", [PART, 3], f32, kind="ExternalOutput")

    with tile.TileContext(nc) as tc:
        with ExitStack() as ctx:
            sb = ctx.enter_context(tc.tile_pool(name="sb", bufs=1))
            ps = ctx.enter_context(tc.tile_pool(name="ps", bufs=1, space="PSUM"))

            joff_t = sb.tile([128, C], i32)
            nc.sync.dma_start(out=joff_t[:], in_=joff[:, :])
            jm = sb.tile([128, C * PART], f32)
            nc.scalar.dma_start(out=jm[:], in_=jmask[:, :])
            cstt = sb.tile([PART, CW], f32)
            nc.scalar.dma_start(out=cstt[:], in_=cst[:, :])
            cnt = cstt[:, 0:1]
            sel = cstt[:, 1:1 + BL]
            ident = cstt[:, 1 + BL:CW]

            # T columns: 0:C gathered g | C:2C g^2 — GpSimd queue holds
            # only these gathers so descriptor gen starts immediately. The
            # last column holds at most LROWS live slots, so its gather
            # carries fewer descriptors and its transfer tail is shorter;
            # the unwritten rows are zeroed so the g^2+matmul stay finite.
            T = sb.tile([128, 2 * C], f32)
            nc.vector.memset(T[:, C - 1:C], 0.0)
            for c in range(C):
                rows = LROWS if c == C - 1 else 128
                nc.gpsimd.indirect_dma_start(
                    out=T[:rows, c:c + 1],
                    out_offset=None,
                    in_=tags[:, :],
                    in_offset=bass.IndirectOffsetOnAxis(
                        ap=joff_t[:rows, c:c + 1], axis=0),
                )

            # ---- gather-independent chain (runs during the gathers) ----
            safe_cnt = sb.tile([PART, 1], f32)
            nc.vector.tensor_scalar_max(out=safe_cnt[:], in0=cnt, scalar1=1.0)
            icnt = sb.tile([PART, 1], f32)
            nc.vector.reciprocal(out=icnt[:], in_=safe_cnt[:])
            stacked = sb.tile([PART, 3], f32)  # pull_p*valid | valid | push row
            nc.vector.tensor_scalar(out=stacked[:, 1:2], in0=cnt, scalar1=0.0,
                                    scalar2=None, op0=Alu.is_gt)
            vb = sb.tile([PART, BL], f32)
            nc.vector.tensor_scalar(out=vb[:], in0=sel,
                                    scalar1=stacked[:, 1:2], scalar2=None,
                                    op0=Alu.mult)
            bf16 = mybir.dt.bfloat16
            vbb = sb.tile([PART, BL], bf16)
            nc.vector.tensor_copy(out=vbb[:], in_=vb[:])

            # per-person (sum g, sum g^2) via accumulating one-hot matmuls
            T3 = T[:].rearrange("p (st c) -> p st c", c=C)
            Pacc = ps.tile([PART, 2], f32, space="PSUM")
            for c in range(C):
                nc.vector.tensor_tensor(out=T[:, C + c:C + c + 1],
                                        in0=T[:, c:c + 1],
                                        in1=T[:, c:c + 1], op=Alu.mult)
                nc.tensor.matmul(out=Pacc[:],
                                 lhsT=jm[:, c * PART:(c + 1) * PART],
                                 rhs=T3[:, :, c],
                                 start=(c == 0), stop=(c == C - 1))

            mean = sb.tile([PART, 1], f32)
            nc.vector.tensor_tensor(out=mean[:], in0=Pacc[:, 0:1], in1=icnt[:],
                                    op=Alu.mult)
            mean2 = sb.tile([PART, 1], f32)
            nc.vector.tensor_tensor(out=mean2[:], in0=mean[:], in1=mean[:],
                                    op=Alu.mult)

            # pull on ACT in parallel with the DVE/PE push-field chain
            a2 = sb.tile([PART, 1], f32)
            nc.scalar.activation(out=a2[:], in_=Pacc[:, 1:2],
                                 func=mybir.ActivationFunctionType.Copy,
                                 scale=icnt[:])
            nc.vector.scalar_tensor_tensor(out=stacked[:, 0:1], in0=a2[:],
                                           scalar=mean2[:], in1=stacked[:, 1:2],
                                           op0=Alu.subtract, op1=Alu.mult)

            meanT = ps.tile([PART, PART], f32, space="PSUM")
            nc.tensor.transpose(out=meanT[:],
                                in_=mean[:].to_broadcast([PART, PART]),
                                identity=ident)
            diff = sb.tile([PART, PART], f32)
            nc.vector.tensor_tensor(out=diff[:],
                                    in0=mean[:].to_broadcast([PART, PART]),
                                    in1=meanT[:], op=Alu.subtract)
            sq = sb.tile([PART, PART], f32)
            nc.vector.tensor_tensor(out=sq[:], in0=diff[:], in1=diff[:],
                                    op=Alu.mult)
            pm = sb.tile([PART, PART], bf16)
            nc.scalar.activation(out=pm[:], in_=sq[:],
                                 func=mybir.ActivationFunctionType.Exp,
                                 scale=-1.0)

            # push rows: (pm @ vb) * vb, row-summed — pm is symmetric, so
            # lhsT=pm gives sum_r pm[q,r]*valid_r*same_image(r,b)
            pvb = ps.tile([PART, BL], f32, space="PSUM")
            nc.tensor.matmul(out=pvb[:], lhsT=pm[:], rhs=vbb[:],
                             start=True, stop=True)
            t1 = sb.tile([PART, BL], f32)
            nc.vector.scalar_tensor_tensor(out=t1[:], in0=pvb[:], scalar=1.0,
                                           in1=vb[:], op0=Alu.mult,
                                           op1=Alu.mult,
                                           accum_out=stacked[:, 2:3])

            nc.sync.dma_start(out=out[:, :], in_=stacked[:])

    nc.compile()
    return nc


def _get_nc():
    if "nc" not in _CACHE:
        _CACHE["nc"] = _build()
    return _CACHE["nc"]


def _make_in_maps(tags: np.ndarray, joints: np.ndarray):
    tags = np.asarray(tags, dtype=np.float32).reshape(B, N)
    joints = np.asarray(joints, dtype=np.int32)

    sel = np.repeat(np.eye(BL, dtype=np.float32), P, axis=0)       # [120, BL]
    ident = np.eye(PART, dtype=np.float32)                         # [120, 120]

    in_maps = []
    for i in range(M):
        t = np.ascontiguousarray(tags[i * BL:(i + 1) * BL].reshape(BL * N, 1))
        sl = joints[i * BL:(i + 1) * BL]  # [BL, P, J, 2]
        vis = sl[..., 1] > 0
        bb, pp, jj = np.nonzero(vis)
        n = bb.size
        assert n <= 128 * (C - 1) + LROWS, f"visible joints {n} exceed capacity"
        tag_idx = (sl[..., 0][bb, pp, jj] + bb * N).astype(np.int32)
        person = (bb * P + pp).astype(np.int32)
        k = np.arange(n)
        prow, pcol = k % 128, k // 128
        joff = np.zeros((128, C), np.int32)
        joff[prow, pcol] = tag_idx
        jmask = np.zeros((128, C * PART), np.float32)
        jmask[prow, pcol * PART + person] = 1.0
        cnt = vis.sum(-1).astype(np.float32).reshape(PART, 1)
        cst = np.concatenate([cnt, sel, ident], axis=1)  # [120, CW]
        in_maps.append({"tags": t, "joff": joff, "jmask": jmask,
                        "cst": np.ascontiguousarray(cst)})
    return in_maps


def _finalize(stacked: np.ndarray):
    # stacked: [PART, 3] per-person (pull_p*valid, valid, push row); the
    # per-image reduction + final normalization run on host.
    red = stacked.astype(np.float64).reshape(BL, P, 3).sum(axis=1)
    pull_sum = red[:, 0]
    nt = red[:, 1]
    push_tot = red[:, 2]
    pull = pull_sum / np.maximum(nt, 1.0)
    denom = np.maximum((nt - 1.0) * nt, 1.0)
    push = np.where(nt > 1.0, (push_tot - nt) / denom * 0.5, 0.0)
    return push.astype(np.float32), pull.astype(np.float32)


def _run(tags, joints, trace=False):
    from concourse.bass_utils import run_bass_kernel_spmd

    nc = _get_nc()
    in_maps = _make_in_maps(tags, joints)
    res = run_bass_kernel_spmd(
        nc, in_maps, core_ids=list(range(M)), trace=trace,
    )
    push = np.empty(B, np.float32)
    pull = np.empty(B, np.float32)
    for i in range(M):
        p, q = _finalize(np.asarray(res.results[i]["out"]))
        push[i * BL:(i + 1) * BL] = p
        pull[i * BL:(i + 1) * BL] = q
    return (push, pull), res.exec_time_ns


def kernel(tags, joints):
    try:
        (push, pull), _ = _run(tags, joints, trace=False)
    except Exception:
        (push, pull), _ = _run(tags, joints, trace=False)
    return push, pull


# revision 16
# speedup vs baseline: 1.1324x; 1.1324x over previous
"""AELoss on 8 TRN2 NeuronCores — visible-packed gather, lean tail.

Front end matches the proven layout: visible joints packed into a
[128, 9] slot grid, one indirect DMA per column (the ~1.04us/instruction
SWDGE descriptor pass serializes on GpSimd and dominates). Everything
else is streamlined around it:
- GpSimd runs ONLY the 9 gathers; iota/identity/selector constants ship
  from the host in one DMA, so descriptor gen starts as soon as the
  offsets land.
- Per-person visible counts also ship from host, so the valid/selector
  algebra runs during the gathers and the accumulating PE matmuls carry
  just 2 columns (sum g, sum g^2) instead of 3.
- The pairwise push field folds its mask and row reduction through one
  PE matmul (pm @ vb, pm is symmetric) and one fused multiply+row-sum.
- The device ships per-image (pull_sum, num_tags, push_tot); the final
  normalization of 4 scalars per core happens on host.
"""

import numpy as np

B = 32
N = 17 * 256 * 256  # 1114112 flattened tag-map size
P = 30              # max people per image
J = 17              # joints per person
M = 8               # cores
BL = B // M         # images per core = 4
PART = BL * P       # person partitions = 120
C = 9               # packed gather columns (capacity 128*9 = 1152 slots)
LROWS = 32          # live rows in the last gather column (max 1054-1024=30)
CW = 1 + BL + PART  # consts width: cnt | sel | identity

_CACHE = {}


def _build():
    from contextlib import ExitStack

    import concourse.bass as bass
    import concourse.tile as tile
    from concourse import bacc, mybir

    f32 = mybir.dt.float32
    i32 = mybir.dt.int32
    Alu = mybir.AluOpType

    nc = bacc.Bacc("TRN2", target_bir_lowering=False, debug=False)

    tags = nc.dram_tensor("tags", [BL * N, 1], f32, kind="ExternalInput")
    joff = nc.dram_tensor("joff", [128, C], i32, kind="ExternalInput")
    jmask = nc.dram_tensor("jmask", [128, C * PART], f32, kind="ExternalInput")
    cst = nc.dram_tensor("cst", [PART, CW], f32, kind="ExternalInput")
    out = nc.dram_tensor("out", [PART, 3], f32, kind="ExternalOutput")

    with tile.TileContext(nc) as tc:
        with ExitStack() as ctx:
            sb = ctx.enter_context(tc.tile_pool(name="sb", bufs=1))
            ps = ctx.enter_context(tc.tile_pool(name="ps", bufs=1, space="PSUM"))

            joff_t = sb.tile([128, C], i32)
            nc.sync.dma_start(out=joff_t[:], in_=joff[:, :])
            jm = sb.tile([128, C * PART], f32)
            nc.scalar.dma_start(out=jm[:], in_=jmask[:, :])
            cstt = sb.tile([PART, CW], f32)
            nc.scalar.dma_start(out=cstt[:], in_=cst[:, :])
            cnt = cstt[:, 0:1]
            sel = cstt[:, 1:1 + BL]
            ident = cstt[:, 1 + BL:CW]

            # T columns: 0:C gathered g | C:2C g^2 — GpSimd queue holds
            # only these gathers so descriptor gen starts immediately.
            # (Shrinking the sparse last column to 32 rows + memset was
            # tried and REGRESSED ~4.6us — keep all columns full-width.)
            T = sb.tile([128, 2 * C], f32)
            for c in range(C):
                nc.gpsimd.indirect_dma_start(
                    out=T[:, c:c + 1],
                    out_offset=None,
                    in_=tags[:, :],
                    in_offset=bass.IndirectOffsetOnAxis(
                        ap=joff_t[:, c:c + 1], axis=0),
                )

            # ---- gather-independent chain (runs during the gathers) ----
            safe_cnt = sb.tile([PART, 1], f32)
            nc.vector.tensor_scalar_max(out=safe_cnt[:], in0=cnt, scalar1=1.0)
            icnt = sb.tile([PART, 1], f32)
            nc.vector.reciprocal(out=icnt[:], in_=safe_cnt[:])
            stacked = sb.tile([PART, 3], f32)  # pull_p*valid | valid | push row
            nc.vector.tensor_scalar(out=stacked[:, 1:2], in0=cnt, scalar1=0.0,
                                    scalar2=None, op0=Alu.is_gt)
            vb = sb.tile([PART, BL], f32)
            nc.vector.tensor_scalar(out=vb[:], in0=sel,
                                    scalar1=stacked[:, 1:2], scalar2=None,
                                    op0=Alu.mult)
            bf16 = mybir.dt.bfloat16
            vbb = sb.tile([PART, BL], bf16)
            nc.vector.tensor_copy(out=vbb[:], in_=vb[:])

            # per-person (sum g, sum g^2) via accumulating one-hot matmuls
            T3 = T[:].rearrange("p (st c) -> p st c", c=C)
            Pacc = ps.tile([PART, 2], f32, space="PSUM")
            for c in range(C):
                nc.vector.tensor_tensor(out=T[:, C + c:C + c + 1],
                                        in0=T[:, c:c + 1],
                                        in1=T[:, c:c + 1], op=Alu.mult)
                nc.tensor.matmul(out=Pacc[:],
                                 lhsT=jm[:, c * PART:(c + 1) * PART],
                                 rhs=T3[:, :, c],
                                 start=(c == 0), stop=(c == C - 1))

            mean = sb.tile([PART, 1], f32)
            nc.vector.tensor_tensor(out=mean[:], in0=Pacc[:, 0:1], in1=icnt[:],
                                    op=Alu.mult)
            mean2 = sb.tile([PART, 1], f32)
            nc.vector.tensor_tensor(out=mean2[:], in0=mean[:], in1=mean[:],
                                    op=Alu.mult)

            # pull on ACT in parallel with the DVE/PE push-field chain
            a2 = sb.tile([PART, 1], f32)
            nc.scalar.activation(out=a2[:], in_=Pacc[:, 1:2],
                                 func=mybir.ActivationFunctionType.Copy,
                                 scale=icnt[:])
            nc.vector.scalar_tensor_tensor(out=stacked[:, 0:1], in0=a2[:],
                                           scalar=mean2[:], in1=stacked[:, 1:2],
                                           op0=Alu.subtract, op1=Alu.mult)

            meanT = ps.tile([PART, PART], f32, space="PSUM")
            nc.tensor.transpose(out=meanT[:],
                                in_=mean[:].to_broadcast([PART, PART]),
                                identity=ident)
            diff = sb.tile([PART, PART], f32)
            nc.vector.tensor_tensor(out=diff[:],
                                    in0=mean[:].to_broadcast([PART, PART]),
                                    in1=meanT[:], op=Alu.subtract)
            sq = sb.tile([PART, PART], f32)
            nc.vector.tensor_tensor(out=sq[:], in0=diff[:], in1=diff[:],
                                    op=Alu.mult)
            pm = sb.tile([PART, PART], bf16)
            nc.scalar.activation(out=pm[:], in_=sq[:],
                                 func=mybir.ActivationFunctionType.Exp,
                                 scale=-1.0)

            # push rows: (pm @ vb) * vb, row-summed — pm is symmetric, so
            # lhsT=pm gives sum_r pm[q,r]*valid_r*same_image(r,b)
            pvb = ps.tile([PART, BL], f32, space="PSUM")
            nc.tensor.matmul(out=pvb[:], lhsT=pm[:], rhs=vbb[:],
                             start=True, stop=True)
            t1 = sb.tile([PART, BL], f32)
            nc.vector.scalar_tensor_tensor(out=t1[:], in0=pvb[:], scalar=1.0,
                                           in1=vb[:], op0=Alu.mult,
                                           op1=Alu.mult,
                                           accum_out=stacked[:, 2:3])

            nc.sync.dma_start(out=out[:, :], in_=stacked[:])

    nc.compile()
    return nc


def _get_nc():
    if "nc" not in _CACHE:
        _CACHE["nc"] = _build()
    return _CACHE["nc"]


def _make_in_maps(tags: np.ndarray, joints: np.ndarray):
    tags = np.asarray(tags, dtype=np.float32).reshape(B, N)
    joints = np.asarray(joints, dtype=np.int32)

    sel = np.repeat(np.eye(BL, dtype=np.float32), P, axis=0)       # [120, BL]
    ident = np.eye(PART, dtype=np.float32)                         # [120, 120]

    in_maps = []
    for i in range(M):
        t = np.ascontiguousarray(tags[i * BL:(i + 1) * BL].reshape(BL * N, 1))
        sl = joints[i * BL:(i + 1) * BL]  # [BL, P, J, 2]
        vis = sl[..., 1] > 0
        bb, pp, jj = np.nonzero(vis)
        n = bb.size
        assert n <= 128 * C, f"visible joints {n} exceed slot capacity {128 * C}"
        tag_idx = (sl[..., 0][bb, pp, jj] + bb * N).astype(np.int32)
        person = (bb * P + pp).astype(np.int32)
        k = np.arange(n)
        prow, pcol = k % 128, k // 128
        joff = np.zeros((128, C), np.int32)
        joff[prow, pcol] = tag_idx
        jmask = np.zeros((128, C * PART), np.float32)
        jmask[prow, pcol * PART + person] = 1.0
        cnt = vis.sum(-1).astype(np.float32).reshape(PART, 1)
        cst = np.concatenate([cnt, sel, ident], axis=1)  # [120, CW]
        in_maps.append({"tags": t, "joff": joff, "jmask": jmask,
                        "cst": np.ascontiguousarray(cst)})
    return in_maps


def _finalize(stacked: np.ndarray):
    # stacked: [PART, 3] per-person (pull_p*valid, valid, push row); the
    # per-image reduction + final normalization run on host.
    red = stacked.astype(np.float64).reshape(BL, P, 3).sum(axis=1)
    pull_sum = red[:, 0]
    nt = red[:, 1]
    push_tot = red[:, 2]
    pull = pull_sum / np.maximum(nt, 1.0)
    denom = np.maximum((nt - 1.0) * nt, 1.0)
    push = np.where(nt > 1.0, (push_tot - nt) / denom * 0.5, 0.0)
    return push.astype(np.float32), pull.astype(np.float32)


def _run(tags, joints, trace=False):
    from concourse.bass_utils import run_bass_kernel_spmd

    nc = _get_nc()
    in_maps = _make_in_maps(tags, joints)
    res = run_bass_kernel_spmd(
        nc, in_maps, core_ids=list(range(M)), trace=trace,
    )
    push = np.empty(B, np.float32)
    pull = np.empty(B, np.float32)
    for i in range(M):
        p, q = _finalize(np.asarray(res.results[i]["out"]))
        push[i * BL:(i + 1) * BL] = p
        pull[i * BL:(i + 1) * BL] = q
    return (push, pull), res.exec_time_ns


def kernel(tags, joints):
    try:
        (push, pull), _ = _run(tags, joints, trace=False)
    except Exception:
        (push, pull), _ = _run(tags, joints, trace=False)
    return push, pull
